# revision 17
# baseline (speedup 1.0000x reference)
"""Trainium2 Bass kernel for 16-head causal MultiHeadAttention.

Problem shapes (hardcoded): x [4, 2048, 1024], Wq/Wk/Wv/Wo [1024, 1024],
bo [1024]. 16 heads, head_dim 64, causal, softmax scale 1/8.

Sharding: batch-major hybrid. Core c owns batch c//2 and head-half c%2
(8 heads = feature slice [512*(c%2), 512*(c%2)+512)). Each core computes
q/k/v for its 8 heads over its batch's 2048 rows, causal attention for
its 8 (batch, head) pairs, and a partial out-projection [1024, 2048].
Host sums the two partials per batch and adds the bias. This cuts
per-core DMA to ~8MB in / 4MB out (vs 16/16 for pure head-parallel).

Device schedule (single pass, no batch loop):
  - input DMAs are split/ordered so the first matmul can start after
    ~1.5MB lands (wq/wk mo=0 slices + first xt row-chunk).
  - q/k projections keep features on partitions (W stationary); v is
    computed directly in natural [row, feat] layout (x stationary) so no
    PE transposes are needed.
  - scores^T [k, q] per head with kT stationary. The contraction dim is
    zero-padded to K=128 (partner-head rows of kT zeroed) so every
    matmul runs in full 128x128 mode - no PE tiling-mode switches.
  - causal trimming: for diagonal 128k-blocks only the valid q-columns
    are streamed/exp'd; a single [128,128] lower-tri mask handles the
    leading triangle.
  - softmax: exp on ACT reads both heads' score PSUM banks in ONE
    activation instruction (halves the ~293ns/instr ACT overhead);
    denominators via a ones-column appended to V (M=65 ctx matmuls);
    reciprocal runs directly on the ctx PSUM denominator row.
  - projection and out-projection matmul groups are interleaved into the
    attention stream as "background work" with requirement-driven
    draining (emission order = engine queue order, so every group is
    force-drained before the first attention op that reads its output),
    keeping the PE dense so HAM stays at K=8/8.

Matmul inputs bf16 (FWL); PSUM accumulation, softmax denominators and
normalization fp32.
"""

import numpy as np

B, S, D, H = 4, 2048, 1024, 16
HD = 64
N_CORES = 8
QC = 512
KC = 128
n_qi = S // QC  # 4 query chunks
n_kc = S // KC  # 16 key chunks (also 128-row chunks for v)
n_rc = 4  # 512-row chunks for projections

_cache = {}


def _build():
    import concourse.bacc as bacc
    import concourse.tile as tile
    from concourse import mybir

    fp32 = mybir.dt.float32
    bf16 = mybir.dt.bfloat16

    nc = bacc.Bacc("TRN2", target_bir_lowering=False)

    xt_d = nc.dram_tensor("xt", [128, 8, S], bf16, kind="ExternalInput")
    wq_d = nc.dram_tensor("wq", [128, 8, 4, 128], bf16, kind="ExternalInput")
    wk_d = nc.dram_tensor("wk", [128, 8, 4, 128], bf16, kind="ExternalInput")
    wv_d = nc.dram_tensor("wv", [128, 8, 512], bf16, kind="ExternalInput")
    wo_d = nc.dram_tensor("wo", [128, 4, 1024], bf16, kind="ExternalInput")
    mask_d = nc.dram_tensor("mask", [128, 2, 128], bf16, kind="ExternalInput")
    out_d = nc.dram_tensor("outp", [128, 8, S], bf16, kind="ExternalOutput")
    out2_d = nc.dram_tensor("outp2", [128, 8, S], bf16, kind="ExternalOutput")

    with tile.TileContext(nc) as tc:
        with (
            tc.tile_pool(name="const", bufs=1) as cpool,
            tc.tile_pool(name="big", bufs=1) as bigpool,
            tc.tile_pool(name="at", bufs=4) as atpool,
            tc.tile_pool(name="ev", bufs=3) as evpool,
            tc.tile_pool(name="sm", bufs=2) as smpool,
            tc.tile_pool(name="ps", bufs=2, space="PSUM") as ps_pool,
            tc.tile_pool(name="pc", bufs=1, space="PSUM") as pc_pool,
            tc.tile_pool(name="pb", bufs=2, space="PSUM") as pb_pool,
        ):
            # ---- static inputs (DMA order gates the pipeline start) ----
            wq_sb = cpool.tile([128, 8, 4, 128], bf16, tag="wq")
            wk_sb = cpool.tile([128, 8, 4, 128], bf16, tag="wk")
            wv_sb = cpool.tile([128, 8, 512], bf16, tag="wv")
            wo_sb = cpool.tile([128, 4, 1024], bf16, tag="wo")
            mask_sb = cpool.tile([128, 2, 128], bf16, tag="mask")
            xt = bigpool.tile([128, 8, S], bf16, tag="xt")
            qT = bigpool.tile([128, 4, S], bf16, tag="qT")  # [d, pair, n] packed
            kTpk = bigpool.tile([128, 4, S], bf16, tag="kTpk")  # packed [d, pair, n]
            ctxT = bigpool.tile([128, 4, S], bf16, tag="ctxT")
            v_aug = bigpool.tile([128, n_kc, 8, 66], bf16, tag="vaug")

            def xt_sl(rc):
                return slice(rc * QC, (rc + 1) * QC)

            nc.sync.dma_start(wq_sb[:, :, 0, :], wq_d[:, :, 0, :])
            nc.sync.dma_start(wk_sb[:, :, 0, :], wk_d[:, :, 0, :])
            nc.sync.dma_start(xt[:, :, xt_sl(0)], xt_d[:, :, xt_sl(0)])
            nc.sync.dma_start(wv_sb[:], wv_d[:])
            nc.gpsimd.dma_start(mask_sb[:], mask_d[:])
            for rc in range(1, n_rc):
                nc.gpsimd.dma_start(xt[:, :, xt_sl(rc)], xt_d[:, :, xt_sl(rc)])
            nc.gpsimd.dma_start(wq_sb[:, :, 1:4, :], wq_d[:, :, 1:4, :])
            nc.gpsimd.dma_start(wk_sb[:, :, 1:4, :], wk_d[:, :, 1:4, :])
            nc.gpsimd.dma_start(wo_sb[:], wo_d[:])

            # ---- work groups: matmuls now, evacuation deferred (finisher)
            # so no engine head-of-line blocks waiting for the group's PSUM
            def projqk_group(w_sb, dst_is_q, p, rc):
                sl = xt_sl(rc)
                ps = pb_pool.tile([128, QC], fp32, tag="pb", name=f"pqk{p}_{rc}")
                for o in range(8):
                    nc.tensor.matmul(
                        ps[:],
                        w_sb[:, o, p, :],
                        xt[:, o, sl],
                        start=(o == 0),
                        stop=(o == 7),
                    )

                def fin():
                    dst = qT if dst_is_q else kTpk
                    nc.vector.tensor_copy(dst[:, p, sl], ps[:])

                return fin

            def projv_group(ci):
                csl = slice(ci * KC, (ci + 1) * KC)
                ps = pb_pool.tile([128, 8, 64], fp32, tag="pb", name=f"pv{ci}")
                for o in range(8):
                    nc.tensor.matmul(
                        ps[:],
                        xt[:, o, csl],
                        wv_sb[:, o, :],
                        start=(o == 0),
                        stop=(o == 7),
                    )

                def fin():
                    nc.vector.tensor_copy(v_aug[:, ci, :, 0:64], ps[:])

                return fin

            def outproj_group(oc, rc, pas):
                sl = xt_sl(rc)
                fo0 = 2 * pas
                dst = out_d if pas == 0 else out2_d
                ps = pb_pool.tile([128, QC], fp32, tag="pb", name=f"po{oc}_{rc}")
                for fo in (fo0, fo0 + 1):
                    nc.tensor.matmul(
                        ps[:],
                        wo_sb[:, fo, oc * 128 : (oc + 1) * 128],
                        ctxT[:, fo, sl],
                        start=(fo == fo0),
                        stop=(fo == fo0 + 1),
                    )

                def fin():
                    ot = evpool.tile([128, QC], bf16, tag="ot", name=f"o{oc}_{rc}")
                    nc.vector.tensor_copy(ot[:], ps[:])
                    nc.sync.dma_start(dst[:, oc, sl], ot[:])

                return fin

            # ---- phase A: minimal prefix before attention can start ----
            # dummy exp so the ~2.7us ACT table load overlaps the input DMA
            warm = smpool.tile([1, 4], fp32, tag="warm")
            nc.vector.memset(warm[:], 0.0)
            nc.scalar.activation(
                warm[:], warm[:], mybir.ActivationFunctionType.Exp, scale=1.0
            )
            projqk_group(wq_sb, True, 0, 0)()
            projqk_group(wk_sb, False, 0, 0)()
            for ci in range(4):
                projv_group(ci)()
            nc.vector.memset(v_aug[:, :, :, 64:65], 1.0)

            # ---- background queue with labeled, requirement-driven drain ----
            bg = []  # list of (label, fn, args)
            for rc in range(1, n_rc):
                bg.append((("qk", 0, rc), projqk_group, (wq_sb, True, 0, rc)))
                bg.append((("qk", 0, rc), projqk_group, (wk_sb, False, 0, rc)))
                for ci in range(4 * rc, 4 * rc + 4):
                    bg.append((("v", ci), projv_group, (ci,)))
            for p in range(1, 4):
                for rc in range(n_rc):
                    bg.append((("qk", p, rc), projqk_group, (wq_sb, True, p, rc)))
                    bg.append((("qk", p, rc), projqk_group, (wk_sb, False, p, rc)))

            emitted = set()
            emitted.add(("qk", 0, 0))
            for ci in range(4):
                emitted.add(("v", ci))
            pending_fin = []  # finisher of the most recent bg group

            def flush_fin():
                while pending_fin:
                    pending_fin.pop(0)()

            def drain_bg(n):
                for _ in range(n):
                    if not bg:
                        flush_fin()
                        return
                    lbl, f, a = bg.pop(0)
                    flush_fin()
                    pending_fin.append(f(*a))
                    emitted.add(lbl)

            def drain_until(lbl):
                while lbl not in emitted:
                    assert bg, f"bg exhausted before {lbl}"
                    l2, f, a = bg.pop(0)
                    flush_fin()
                    pending_fin.append(f(*a))
                    emitted.add(l2)
                flush_fin()

            # ---- attention ----
            for p in range(4):
                for qi in range(n_qi):
                    drain_until(("qk", p, qi))
                    drain_until(("v", min(4 * qi + 3, n_kc - 1)))
                    q0 = qi * QC
                    kc_hi = 4 * qi + 4
                    pc = pc_pool.tile(
                        [128, 2, QC], fp32, tag="pc", name=f"pc_{p}_{qi}"
                    )
                    def w0_of(kc):
                        j = kc - 4 * qi
                        return 128 * j if j > 0 else 0

                    for kc0 in range(0, kc_hi, 2):
                        pair = (kc0, kc0 + 1)
                        pss, ats = {}, {}
                        for kc in pair:
                            w0 = w0_of(kc)
                            ksl = slice(kc * KC, (kc + 1) * KC)
                            ps = ps_pool.tile(
                                [128, 2, QC], fp32, tag="ps", name=f"ps{p}_{qi}_{kc}"
                            )
                            pss[kc] = ps
                            for h in range(2):
                                hs = slice(64 * h, 64 * h + 64)
                                nc.tensor.matmul(
                                    ps[:, h, w0:QC],
                                    kTpk[hs, p, ksl],
                                    qT[hs, p, q0 + w0 : q0 + QC],
                                    start=True,
                                    stop=True,
                                )
                        for kc in pair:
                            w0 = w0_of(kc)
                            at = atpool.tile(
                                [128, 2, QC], bf16, tag="at", name=f"at{p}_{qi}_{kc}"
                            )
                            ats[kc] = at
                            nc.scalar.activation(
                                at[:, :, w0:QC],
                                pss[kc][:, :, w0:QC],
                                mybir.ActivationFunctionType.Exp,
                                scale=0.125,
                            )
                        for kc in pair:
                            w0 = w0_of(kc)
                            if kc - 4 * qi >= 0:
                                nc.vector.tensor_mul(
                                    ats[kc][:, :, w0 : w0 + 128],
                                    ats[kc][:, :, w0 : w0 + 128],
                                    mask_sb[:],
                                )
                        for kc in pair:
                            w0 = w0_of(kc)
                            for h in range(2):
                                nc.tensor.matmul(
                                    pc[0:65, h, w0:QC],
                                    v_aug[:, kc, 2 * p + h, 0:65],
                                    ats[kc][:, h, w0:QC],
                                    start=(kc == 0),
                                    stop=(kc == kc_hi - 1),
                                )
                        if kc0 + 1 < 4 * qi:
                            drain_bg(1)
                    # ---- normalization ----
                    # single fast cast to SBUF releases the ctx PSUM banks;
                    # recip/broadcast/muls then run off the critical path
                    qsl = slice(q0, q0 + QC)
                    cu = smpool.tile([65, 2, QC], fp32, tag="cu", name=f"cu{p}_{qi}")
                    nc.scalar.copy(cu[:], pc[0:65, :, :])
                    # custom-DVE recip needs a base-partition-0 input AP
                    rec = smpool.tile([1, 2, QC], fp32, tag="rec", name=f"re{p}_{qi}")
                    nc.vector.tensor_copy(rec[:], cu[64:65, :, :])
                    rrec = smpool.tile([1, 2, QC], fp32, tag="rrec", name=f"rr{p}_{qi}")
                    nc.vector.reciprocal_approx_fast(rrec[:], rec[:])
                    rb = smpool.tile([64, 2, QC], fp32, tag="rb", name=f"rb{p}_{qi}")
                    nc.gpsimd.partition_broadcast(rb[:], rrec[0:1, :, :])
                    nc.vector.tensor_mul(
                        ctxT[0:64, p, qsl], cu[0:64, 0, :], rb[:, 0, :]
                    )
                    nc.vector.tensor_mul(
                        ctxT[64:128, p, qsl], cu[0:64, 1, :], rb[:, 1, :]
                    )
                    drain_bg(2)
                    if p == 1 and qi == 3:
                        for rc in range(n_rc):
                            for oc in range(8):
                                bg.append(
                                    (("o1", oc, rc), outproj_group, (oc, rc, 0))
                                )
                    if p == 3 and qi < 3:
                        for oc in range(8):
                            bg.append((("o2", oc, qi), outproj_group, (oc, qi, 1)))
            # ---- drain remaining background + final out-projection ----
            drain_bg(len(bg))
            flush_fin()
            prev = None
            for oc in range(8):
                f = outproj_group(oc, 3, 1)
                if prev is not None:
                    prev()
                prev = f
            prev()

    nc.compile()
    return nc


def _prep_inputs(x, Wq, Wk, Wv, Wo, bo):
    import ml_dtypes

    bf = ml_dtypes.bfloat16

    x = np.ascontiguousarray(np.asarray(x, dtype=np.float32))
    Wq = np.asarray(Wq, dtype=np.float32)
    Wk = np.asarray(Wk, dtype=np.float32)
    Wv = np.asarray(Wv, dtype=np.float32)
    Wo = np.asarray(Wo, dtype=np.float32)

    karr = np.arange(128)[:, None]
    qarr = np.arange(128)[None, :]
    mask = (karr <= qarr).astype(bf)  # [128 k, 128 q]
    mask2 = np.ascontiguousarray(np.stack([mask, mask], axis=1))  # [128, 2, 128]

    # per batch: xt[p, o, n] = x[b, n, 128o+p]
    xts = []
    for b in range(B):
        xts.append(
            np.ascontiguousarray(
                x[b].T.reshape(8, 128, S).transpose(1, 0, 2).astype(bf)
            )
        )

    in_maps = []
    for c in range(N_CORES):
        b = c // 2
        fh = c % 2
        fsl = slice(fh * 512, fh * 512 + 512)

        def wqk(W):
            # wq[p, o, mo, m] = W[fh*512 + 128*mo + m, 128*o + p]
            Ws = W[fsl, :]  # [512, 1024]
            return np.ascontiguousarray(
                Ws.reshape(4, 128, 8, 128).transpose(3, 2, 0, 1).astype(bf)
            )

        # wv[p, o, f] = Wv[fh*512 + f, 128*o + p]
        wv = np.ascontiguousarray(
            Wv[fsl, :].reshape(512, 8, 128).transpose(2, 1, 0).astype(bf)
        )
        # wo[p, fo, m] = Wo[m, fh*512 + 128*fo + p]
        wo = np.ascontiguousarray(
            Wo[:, fsl].reshape(1024, 4, 128).transpose(2, 1, 0).astype(bf)
        )
        in_maps.append(
            {
                "xt": xts[b],
                "wq": wqk(Wq),
                "wk": wqk(Wk),
                "wv": wv,
                "wo": wo,
                "mask": mask2,
            }
        )
    return in_maps


def _run(in_maps, trace=False):
    from concourse.bass_utils import run_bass_kernel_spmd

    if "nc" not in _cache:
        _cache["nc"] = _build()
    return run_bass_kernel_spmd(
        _cache["nc"], in_maps, core_ids=list(range(N_CORES)), trace=trace
    )


def kernel(x, Wq, Wk, Wv, Wo, bo, _trace=False):
    in_maps = _prep_inputs(x, Wq, Wk, Wv, Wo, bo)
    res = _run(in_maps, trace=_trace)
    bo32 = np.asarray(bo, dtype=np.float32)
    out = np.empty((B, S, D), dtype=np.float32)
    for b in range(B):
        # partial [128, 8, S] -> [1024, S]; out rows = partial.T
        acc = np.zeros((128, 8, S), dtype=np.float32)
        for c in (2 * b, 2 * b + 1):
            acc += res.results[c]["outp"].astype(np.float32)
            acc += res.results[c]["outp2"].astype(np.float32)
        acc = acc.transpose(1, 0, 2).reshape(D, S)
        out[b] = acc.T + bo32[None, :]
    if _trace:
        kernel.last_exec_time_ns = res.exec_time_ns
    return out
